# revision 28
# baseline (speedup 1.0000x reference)
"""Trainium2 Bass kernel for nn_AdjacencyMatrix (gnn_message_passing).

Math (per reference):
  xs    = x.sum(c)                                  [V,B,T]
  z     = conv1d(xs, w[O,1,K], pad=2) + b           [V,B,O,T]
  conv  = selu(z)
  s     = conv.mean(T)                              [V,B,O]
  gate  = sigmoid(W2 @ relu(W1 @ s + b1) + b2)      per-vertex SE
  comp  = gate * s            (gate is T-constant, so (conv*gate).mean(T) == gate*s)
  aw[f,g,b] = selu(af[f,b] + at[g,b]),  af = comp@wA, at = comp@wB
  sm    = softmax_f(aw)
  out[g]= sum_f sm[f,g] * conv[f]                   [V,B,O,T]

Strategy: data-parallel over B across 8 cores (B_local=4), no collectives.
Per core, per b:
  - xsum via ones-matmul (contract c on PE), bf16
  - im2col [ (f,k)=40, T ] built with 5 SBUF->SBUF DMAs
  - conv as block-diag matmul: lhsT[(f,k),(f,oc)] -> psum z[(f,oc), t], o = 4*oc+j
  - SELU/s stored exactly:  ez=Exp(z+cb) [ACT], t1=(min(ez,1)*alpha - alpha) [DVE ts],
    r=Relu(z+cb) [ACT or DVE], stored = t1 + r [DVE stt, accum_out -> T-sums]
  - SE + attention on tiny tensors (PE matmuls with host-packed block-diag weights)
  - mix: lhsT_mix = kron(s*sm, I16) built by broadcast-DMA + mask-mul;
    out[(g,oc),t] = lhsT_mix.T @ stored ; psum -> SBUF copy (ACT/DVE split) -> DMA out
"""

import os
from contextlib import ExitStack

import numpy as np
import ml_dtypes

import concourse.bass as bass
import concourse.tile as tile
from concourse import bacc, mybir
from concourse.bass_utils import run_bass_kernel_spmd

F32 = mybir.dt.float32
BF16 = mybir.dt.bfloat16
AF = mybir.ActivationFunctionType
ALU = mybir.AluOpType

V, B, C, T = 8, 32, 8, 4096
O, K, H = 64, 5, 16
NCORES = 8
BL = B // NCORES  # 4 batches per core
SELU_S = 1.0507009873554805
SELU_A = 1.6732632423543772
# stored = selu(z)/SELU_S = relu(z) + SELU_A*(min(e^z,1) - 1)

# chunking of T for psum tiles (3 banks = 1536 f32)
CHUNKS = [(0, 1536), (1536, 1536), (3072, 1024)]
NCH = len(CHUNKS)

# engine-balance knobs
RELU_DVE_FRAC = 0.0   # fraction of relu tiles on DVE (rest ACT)
COPY_ACT_FRAC = 0.5   # fraction of output-copy tiles on ACT (rest DVE)

bf16 = ml_dtypes.bfloat16


def _host_consts(conv_w, conv_b, se_w1, se_b1, se_w2, se_b2, attn_w):
    """Pack weight-derived constants for the kernel layouts. o = 4*oc + j."""
    cw = conv_w.astype(np.float64)  # [O,1,K]
    cb = conv_b.astype(np.float64)  # [O]

    ones64 = np.zeros((64, 8), np.float32)  # rows (v,c) -> col v
    for p in range(64):
        ones64[p, p // 8] = 1.0

    # im2col row order: k-major (row = k*8 + f)
    Lconv = np.zeros((4, 40, 128), np.float64)
    for j in range(4):
        for f in range(8):
            for k in range(K):
                for oc in range(16):
                    Lconv[j, k * 8 + f, f * 16 + oc] = cw[4 * oc + j, 0, k]

    br = np.zeros((128, 4), np.float64)  # conv bias at partition (f,oc), col j
    for f in range(8):
        for oc in range(16):
            for j in range(4):
                br[f * 16 + oc, j] = cb[4 * oc + j]

    sT = SELU_S / T
    L1 = np.zeros((4, 128, 128), np.float64)
    for j in range(4):
        for v in range(8):
            for oc in range(16):
                for hh in range(H):
                    L1[j, v * 16 + oc, v * 16 + hh] = se_w1[v, hh, 4 * oc + j] * sT
    b1c = np.zeros((128, 1), np.float64)
    for v in range(8):
        for hh in range(H):
            b1c[v * 16 + hh, 0] = se_b1[v, hh]

    L2 = np.zeros((4, 128, 128), np.float64)
    for jp in range(4):
        for v in range(8):
            for hh in range(H):
                for oc in range(16):
                    L2[jp, v * 16 + hh, v * 16 + oc] = se_w2[v, 4 * oc + jp, hh]
    nb2 = np.zeros((128, 4), np.float64)
    for v in range(8):
        for oc in range(16):
            for jp in range(4):
                nb2[v * 16 + oc, jp] = -se_b2[v, 4 * oc + jp]

    # LA: af weights (attn_w[:64]); LB: at weights (attn_w[64:])
    LA = np.zeros((4, 128, 8), np.float64)
    LB = np.zeros((4, 128, 8), np.float64)
    for jp in range(4):
        for v in range(8):
            for oc in range(16):
                LA[jp, v * 16 + oc, v] = attn_w[4 * oc + jp]
                LB[jp, v * 16 + oc, v] = attn_w[64 + 4 * oc + jp]
    ones18 = np.ones((1, 8), np.float32)
    eye8 = np.eye(8, dtype=np.float32)
    eye8b = np.eye(8, dtype=np.float32)
    sel8 = np.zeros((8, 128), np.float64)
    for f in range(8):
        sel8[f, f * 16:(f + 1) * 16] = 1.0

    kmask = np.zeros((128, 128), np.float64)
    for f in range(8):
        for oc in range(16):
            for g in range(8):
                kmask[f * 16 + oc, g * 16 + oc] = SELU_S

    return {
        "ones64": ones64,
        "lconv": Lconv.astype(bf16),
        "br": br.astype(np.float32),
        "l1": L1.astype(bf16),
        "b1c": b1c.astype(np.float32),
        "l2": L2.astype(bf16),
        "nb2": nb2.astype(np.float32),
        "la": LA.astype(bf16),
        "lb": LB.astype(bf16),
        "ones18": ones18,
        "eye8": eye8,
        "eye8b": eye8b.astype(bf16),
        "sel8": sel8.astype(bf16),
        "kmask": kmask.astype(bf16),
    }


def _build_graph():
    nc = bacc.Bacc("TRN2", target_bir_lowering=False, debug=False)

    x_d = nc.dram_tensor("x", [V, BL, C, T], F32, kind="ExternalInput").ap()
    ones64_d = nc.dram_tensor("ones64", [64, 8], F32, kind="ExternalInput").ap()
    lconv_d = nc.dram_tensor("lconv", [4, 40, 128], BF16, kind="ExternalInput").ap()
    br_d = nc.dram_tensor("br", [128, 4], F32, kind="ExternalInput").ap()
    l1_d = nc.dram_tensor("l1", [4, 128, 128], BF16, kind="ExternalInput").ap()
    b1c_d = nc.dram_tensor("b1c", [128, 1], F32, kind="ExternalInput").ap()
    l2_d = nc.dram_tensor("l2", [4, 128, 128], BF16, kind="ExternalInput").ap()
    nb2_d = nc.dram_tensor("nb2", [128, 4], F32, kind="ExternalInput").ap()
    la_d = nc.dram_tensor("la", [4, 128, 8], BF16, kind="ExternalInput").ap()
    lb_d = nc.dram_tensor("lb", [4, 128, 8], BF16, kind="ExternalInput").ap()
    ones18_d = nc.dram_tensor("ones18", [1, 8], F32, kind="ExternalInput").ap()
    eye8_d = nc.dram_tensor("eye8", [8, 8], F32, kind="ExternalInput").ap()
    eye8b_d = nc.dram_tensor("eye8b", [8, 8], BF16, kind="ExternalInput").ap()
    sel8_d = nc.dram_tensor("sel8", [8, 128], BF16, kind="ExternalInput").ap()
    kmask_d = nc.dram_tensor("kmask", [128, 128], BF16, kind="ExternalInput").ap()
    out_d = nc.dram_tensor("out", [V, BL, O, T], F32, kind="ExternalOutput").ap()

    with ExitStack() as ctx:
        tc = ctx.enter_context(tile.TileContext(nc))
        cpool = ctx.enter_context(tc.tile_pool(name="consts", bufs=1))
        work = ctx.enter_context(tc.tile_pool(name="work", bufs=2))
        spool = ctx.enter_context(tc.tile_pool(name="small", bufs=2))
        stpool = ctx.enter_context(tc.tile_pool(name="stats", bufs=2))
        pbig = ctx.enter_context(tc.tile_pool(name="pbig", bufs=2, space="PSUM"))
        psm = ctx.enter_context(tc.tile_pool(name="psm", bufs=2, space="PSUM"))

        # ---- load constants to SBUF
        ones64_s = cpool.tile([64, 8], F32, tag="c0")
        nc.sync.dma_start(ones64_s[:], ones64_d[:])
        lconv_s = cpool.tile([40, 4, 128], BF16, tag="c1")
        nc.sync.dma_start(lconv_s[:], lconv_d.rearrange("j k m -> k j m"))
        br_s = cpool.tile([128, 4], F32, tag="c2")
        nc.sync.dma_start(br_s[:], br_d[:])
        l1_s = cpool.tile([128, 4, 128], BF16, tag="c3")
        nc.sync.dma_start(l1_s[:], l1_d.rearrange("j k m -> k j m"))
        b1c_s = cpool.tile([128, 1], F32, tag="c4")
        nc.sync.dma_start(b1c_s[:], b1c_d[:])
        l2_s = cpool.tile([128, 4, 128], BF16, tag="c5")
        nc.sync.dma_start(l2_s[:], l2_d.rearrange("j k m -> k j m"))
        nb2_s = cpool.tile([128, 4], F32, tag="c6")
        nc.sync.dma_start(nb2_s[:], nb2_d[:])
        la_s = cpool.tile([128, 4, 8], BF16, tag="c7")
        nc.sync.dma_start(la_s[:], la_d.rearrange("j k m -> k j m"))
        lb_s = cpool.tile([128, 4, 8], BF16, tag="c7b")
        nc.sync.dma_start(lb_s[:], lb_d.rearrange("j k m -> k j m"))
        ones18_s = cpool.tile([1, 8], F32, tag="c8")
        nc.sync.dma_start(ones18_s[:], ones18_d[:])
        eye8_s = cpool.tile([8, 8], F32, tag="c10")
        nc.sync.dma_start(eye8_s[:], eye8_d[:])
        eye8b_s = cpool.tile([8, 8], BF16, tag="c11")
        nc.sync.dma_start(eye8b_s[:], eye8b_d[:])
        sel8_s = cpool.tile([8, 128], BF16, tag="c12")
        nc.sync.dma_start(sel8_s[:], sel8_d[:])
        kmask_s = cpool.tile([128, 128], BF16, tag="c9")
        nc.sync.dma_start(kmask_s[:], kmask_d[:])

        x_r = x_d.rearrange("v b c t -> b v c t")  # [BL, 8, 8, T]
        # out view: [b, j, g, oc, t]
        out_r = out_d.rearrange("g b (oc j) t -> b j g oc t", j=4)

        cnt_relu = 0
        cnt_copy = 0
        for b in range(BL):
            # ---- load x_b and sum over c via ones-matmul
            x_b = work.tile([64, T], F32, tag="xb")
            nc.sync.dma_start(x_b[:], x_r[b])
            xsum_b = work.tile([8, T], BF16, tag="xsum")
            for tch in range(8):
                ps_x = psm.tile([128, 512], F32, tag="sm")
                nc.tensor.matmul(
                    ps_x[:8, :], ones64_s[:], x_b[:, tch * 512:(tch + 1) * 512],
                    start=True, stop=True)
                nc.vector.tensor_copy(
                    xsum_b[:, tch * 512:(tch + 1) * 512], ps_x[:8, :])

            # ---- im2col [40, T]: row k*8+f holds xsum[f, t+k-2]
            i2c = work.tile([40, T], BF16, tag="i2c")
            nc.vector.memset(i2c[:], 0.0)
            for k in range(K):
                lo = max(0, 2 - k)
                hi = T + min(0, 2 - k)
                nc.sync.dma_start(
                    i2c[k * 8:(k + 1) * 8, lo:hi],
                    xsum_b[:, lo + k - 2:hi + k - 2])

            # ---- conv + selu per j
            store_b = work.tile([128, 4, T], BF16, tag="store")
            stats_b = stpool.tile([128, 4 * NCH], F32, tag="st")
            for j in range(4):
                for ci, (off, clen) in enumerate(CHUNKS):
                    ps_c = pbig.tile([128, 1536], F32, tag="big")
                    for s0 in range(0, clen, 512):
                        nc.tensor.matmul(
                            ps_c[:, s0:s0 + 512],
                            lconv_s[:, j, :],
                            i2c[:, off + s0:off + s0 + 512],
                            start=True, stop=True)
                    zvw = ps_c[:, :clen]
                    ez = spool.tile([128, 1536], BF16, tag="ez")
                    nc.scalar.activation(
                        ez[:, :clen], zvw, AF.Exp, bias=br_s[:, j:j + 1])
                    t1 = spool.tile([128, 1536], BF16, tag="t1")
                    nc.vector.tensor_scalar(
                        t1[:, :clen], ez[:, :clen],
                        1.0, float(SELU_A), op0=ALU.min, op1=ALU.mult)
                    # t1 = min(ez,1)*alpha ; stored = t1 + r - alpha
                    rr = spool.tile([128, 1536], BF16, tag="rr")
                    cnt_relu += 1
                    if (cnt_relu * RELU_DVE_FRAC) % 1 < RELU_DVE_FRAC:
                        nc.vector.tensor_scalar(
                            rr[:, :clen], zvw, br_s[:, j:j + 1], 0.0,
                            op0=ALU.add, op1=ALU.max)
                    else:
                        nc.scalar.activation(
                            rr[:, :clen], zvw, AF.Relu, bias=br_s[:, j:j + 1])
                    nc.vector.scalar_tensor_tensor(
                        store_b[:, j, off:off + clen],
                        t1[:, :clen], float(-SELU_A), rr[:, :clen],
                        op0=ALU.add, op1=ALU.add,
                        accum_out=stats_b[:, j * NCH + ci:j * NCH + ci + 1])

            # ---- SE + attention (tiny)
            sums_b = stpool.tile([128, 4], F32, tag="sums")
            for j in range(4):
                nc.vector.reduce_sum(
                    sums_b[:, j:j + 1], stats_b[:, j * NCH:(j + 1) * NCH],
                    axis=mybir.AxisListType.X)
            # s_true(bf16) = sums * (SELU_S/T)  [scale folded into L1 for matmul]
            st1b = stpool.tile([128, 4], BF16, tag="st1b")
            nc.vector.tensor_copy(st1b[:], sums_b[:])
            st1f = stpool.tile([128, 4], F32, tag="st1f")
            nc.vector.tensor_scalar(
                st1f[:], sums_b[:], float(SELU_S / T), None, op0=ALU.mult)

            ps_h = psm.tile([128, 512], F32, tag="sm")
            for j in range(4):
                nc.tensor.matmul(
                    ps_h[:, 0:1], l1_s[:, j, :], st1b[:, j:j + 1],
                    start=(j == 0), stop=(j == 3))
            h_sb = stpool.tile([128, 1], BF16, tag="h")
            nc.scalar.activation(
                h_sb[:], ps_h[:, 0:1], AF.Relu, bias=b1c_s[:, 0:1])

            ps_g = psm.tile([128, 512], F32, tag="sm")
            for jp in range(4):
                nc.tensor.matmul(
                    ps_g[:, jp:jp + 1], l2_s[:, jp, :], h_sb[:],
                    start=True, stop=True)
            eg = stpool.tile([128, 4], F32, tag="eg")
            for jp in range(4):
                nc.scalar.activation(
                    eg[:, jp:jp + 1], ps_g[:, jp:jp + 1], AF.Exp,
                    bias=nb2_s[:, jp:jp + 1], scale=-1.0)
            gp1 = stpool.tile([128, 4], F32, tag="gp1")
            nc.vector.tensor_scalar(gp1[:], eg[:], 1.0, None, op0=ALU.add)
            gate = stpool.tile([128, 4], F32, tag="gate")
            nc.vector.reciprocal(gate[:], gp1[:])
            comp = stpool.tile([128, 4], BF16, tag="comp")
            nc.vector.tensor_tensor(comp[:], gate[:], st1f[:], op=ALU.mult)

            ps_af = psm.tile([128, 512], F32, tag="sm")
            ps_at = psm.tile([128, 512], F32, tag="sm")
            for jp in range(4):
                nc.tensor.matmul(
                    ps_af[:8, 0:1], la_s[:, jp, :], comp[:, jp:jp + 1],
                    start=(jp == 0), stop=(jp == 3))
                nc.tensor.matmul(
                    ps_at[:8, 0:1], lb_s[:, jp, :], comp[:, jp:jp + 1],
                    start=(jp == 0), stop=(jp == 3))
            af_sb = stpool.tile([8, 1], F32, tag="afc")
            nc.vector.tensor_copy(af_sb[:], ps_af[:8, 0:1])
            at_sb = stpool.tile([8, 1], F32, tag="atc")
            nc.vector.tensor_copy(at_sb[:], ps_at[:8, 0:1])
            # af row [1,8] via PE transpose (lhsT=af col, rhs=I8)
            ps_afr = psm.tile([128, 512], F32, tag="sm")
            nc.tensor.matmul(ps_afr[:1, 0:8], af_sb[:], eye8_s[:],
                             start=True, stop=True)
            af_row = stpool.tile([1, 8], F32, tag="afrow")
            nc.vector.tensor_copy(af_row[:], ps_afr[:1, 0:8])
            ps_zA = psm.tile([128, 512], F32, tag="sm")
            nc.tensor.matmul(ps_zA[:8, 0:8], ones18_s[:], af_row[:],
                             start=True, stop=True)
            zaw = stpool.tile([8, 8], F32, tag="zaw")
            nc.vector.tensor_scalar(
                zaw[:], ps_zA[:8, 0:8], at_sb[:], None, op0=ALU.add)
            # selu/s on zaw: q = relu(zaw) + alpha*(min(e^zaw,1)-1)
            ezw = stpool.tile([8, 8], F32, tag="ezw")
            nc.scalar.activation(ezw[:], zaw[:], AF.Exp)
            t1w = stpool.tile([8, 8], F32, tag="t1w")
            nc.vector.tensor_scalar(
                t1w[:], ezw[:], 1.0, float(SELU_A), op0=ALU.min, op1=ALU.mult)
            rw = stpool.tile([8, 8], F32, tag="rw")
            nc.scalar.activation(rw[:], zaw[:], AF.Relu)
            qw = stpool.tile([8, 8], F32, tag="qw")
            nc.vector.scalar_tensor_tensor(
                qw[:], t1w[:], float(-SELU_A), rw[:], op0=ALU.add, op1=ALU.add)
            # softmax over f (free dim), logits = SELU_S * qw
            mx = stpool.tile([8, 1], F32, tag="mx")
            nc.vector.reduce_max(mx[:], qw[:], axis=mybir.AxisListType.X)
            qs = stpool.tile([8, 8], F32, tag="qs")
            nc.vector.tensor_scalar(
                qs[:], qw[:], mx[:], float(SELU_S),
                op0=ALU.subtract, op1=ALU.mult)
            eq = stpool.tile([8, 8], F32, tag="eq")
            nc.scalar.activation(eq[:], qs[:], AF.Exp)
            ssum = stpool.tile([8, 1], F32, tag="ssum")
            nc.vector.reduce_sum(ssum[:], eq[:], axis=mybir.AxisListType.X)
            rsum = stpool.tile([8, 1], F32, tag="rsum")
            nc.vector.reciprocal(rsum[:], ssum[:])
            sm_b = stpool.tile([8, 8], BF16, tag="smb")
            nc.vector.tensor_scalar(
                sm_b[:], eq[:], rsum[:], None, op0=ALU.mult)

            # ---- kron(s*sm, I16): PE transpose -> PE partition-broadcast ->
            #      per-g column expand via per-partition-scalar multiplies
            ps_smT = psm.tile([128, 512], F32, tag="sm")
            nc.tensor.matmul(ps_smT[:8, 0:8], sm_b[:], eye8b_s[:],
                             start=True, stop=True)
            smT = stpool.tile([8, 8], BF16, tag="smT")
            nc.vector.tensor_copy(smT[:], ps_smT[:8, 0:8])
            ps_bc = psm.tile([128, 512], F32, tag="sm")
            nc.tensor.matmul(ps_bc[:, 0:8], sel8_s[:], smT[:],
                             start=True, stop=True)
            smbc8 = stpool.tile([128, 8], F32, tag="smbc8")
            nc.vector.tensor_copy(smbc8[:], ps_bc[:, 0:8])
            lmix = stpool.tile([128, 128], BF16, tag="lmix")
            for g in range(8):
                nc.vector.tensor_scalar(
                    lmix[:, g * 16:(g + 1) * 16],
                    kmask_s[:, g * 16:(g + 1) * 16],
                    smbc8[:, g:g + 1], None, op0=ALU.mult)

            # ---- mix + output
            for j in range(4):
                for (off, clen) in CHUNKS:
                    ps_m = pbig.tile([128, 1536], F32, tag="big")
                    for s0 in range(0, clen, 512):
                        nc.tensor.matmul(
                            ps_m[:, s0:s0 + 512],
                            lmix[:],
                            store_b[:, j, off + s0:off + s0 + 512],
                            start=True, stop=True)
                    stg = spool.tile([128, 1536], F32, tag="stg")
                    cnt_copy += 1
                    if (cnt_copy * COPY_ACT_FRAC) % 1 >= COPY_ACT_FRAC:
                        nc.vector.tensor_copy(stg[:, :clen], ps_m[:, :clen])
                    else:
                        nc.scalar.activation(
                            stg[:, :clen], ps_m[:, :clen], AF.Copy)
                    nc.sync.dma_start(
                        out_r[b, j][:, :, off:off + clen], stg[:, :clen])
    return nc


_CACHE = {}


def _get_nc():
    if "nc" not in _CACHE:
        nc = _build_graph()
        nc.compile()
        _CACHE["nc"] = nc
    return _CACHE["nc"]


def _ensure_ntff_hook():
    """The image's antenv lacks axon_hooks; synthesize it so trace=True works."""
    import sys
    import types
    try:
        from antenv import axon_hooks  # noqa: F401
        return
    except ImportError:
        pass
    mod = types.ModuleType("antenv.axon_hooks")
    _state = {"hook": None}
    mod.set_axon_ntff_profile_hook = lambda h: _state.__setitem__("hook", h)
    mod.get_axon_ntff_profile_hook = lambda: _state["hook"]
    sys.modules["antenv.axon_hooks"] = mod
    import antenv
    antenv.axon_hooks = mod
    try:
        from trn_agent_boot.trn_boot import _ntff_profile_via_ctypes
        mod.set_axon_ntff_profile_hook(
            _ntff_profile_via_ctypes("/opt/axon/libaxon_pjrt.so"))
    except Exception:
        pass


def kernel(x, conv_w, conv_b, se_w1, se_b1, se_w2, se_b2, attn_w, _profile=False):
    if _profile:
        _ensure_ntff_hook()
    x = np.asarray(x, np.float32)
    consts = _host_consts(
        np.asarray(conv_w), np.asarray(conv_b), np.asarray(se_w1),
        np.asarray(se_b1), np.asarray(se_w2), np.asarray(se_b2),
        np.asarray(attn_w))
    nc = _get_nc()
    in_maps = []
    for i in range(NCORES):
        m = dict(consts)
        m["x"] = np.ascontiguousarray(x[:, i * BL:(i + 1) * BL])
        in_maps.append(m)
    res = run_bass_kernel_spmd(
        nc, in_maps, core_ids=list(range(NCORES)), trace=_profile)
    out = np.concatenate([r["out"] for r in res.results], axis=1)
    if _profile:
        return out, res
    return out
